# revision 21
# baseline (speedup 1.0000x reference)
"""Trainium2 Bass kernel for DGP-RF embeddings (segment_reduce).

Reference computation (N=500000, D_IN=128, R=256, D_OUT=64, U=10000):
    m0 = X @ Wmu0                      # [N, R]
    v0 = (X*X) @ exp(Wlv0)             # [N, R]
    gate = m0 > 0 ; m = m0*gate ; v = v0*gate
    M1 = m @ Wmu1                      # [N, 64]
    V1 = v @ (Wmu1^2 + exp(Wlv1)) + (m*m) @ exp(Wlv1)
    inv = 1/max(V1, eps)
    var_inv_sum = segment_sum(inv, X_idx, U) + eps
    mean_sum    = segment_sum(M1*inv, X_idx, U)
    emb_var  = 1/var_inv_sum ; emb_mean = mean_sum * emb_var

Strategy (8 cores, data-parallel over rows):
  - Host: shard rows, pre-transpose X to [128, rows] bf16, precompute X^2,
    precompute exp() weight combos, and build the one-hot segment-selector
    matrix S (rows x 128-seg window per group) from the sorted X_idx.
  - Device per 512-row chunk: L0 matmuls (weights stationary, transposed
    activations), ReLU gate (ACT relu / ACT square / DVE copy_predicated),
    L1 matmuls (data stationary -> natural-layout [rows, 64] outputs),
    reciprocal + weighting on DVE, then a segment-reduce matmul with the
    host-built one-hot S as the stationary operand, accumulating 128-segment
    windows in PSUM, flushed per group to DRAM.
  - Host: add per-group 128-seg slabs into full [U, 128] per core, sum
    cores, final divide.
"""

import os
import sys

sys.path.insert(0, "/opt/trn_rl_repo")

import numpy as np
import ml_dtypes

import concourse.bass as bass
import concourse.bacc as bacc
import concourse.mybir as mybir
import concourse.tile as tile
from contextlib import ExitStack

BF16 = ml_dtypes.bfloat16

N, D_IN, R, D_OUT, U = 500000, 128, 256, 64, 10000
EPS = 1e-8
N_CORES = 8
P = 128
F = 512                      # rows per chunk
SHARD = N // N_CORES         # 62500


def _choose_grouping(idx_shards, group_subs):
    """Check that every group of `group_subs` subchunks spans < 128 segments.

    idx_shards: list of per-core int arrays (sorted), real rows only.
    Returns True if grouping is valid.
    """
    rows_per_group = group_subs * P
    for idx in idx_shards:
        n = len(idx)
        for start in range(0, n, rows_per_group):
            seg = idx[start : start + rows_per_group]
            if len(seg) and seg[-1] - seg[0] >= P:
                return False
    return True


def _build_program(n_chunks, chunks_per_group, n_groups):
    dt = mybir.dt
    rows_pad = n_chunks * F
    nc = bacc.Bacc()

    xin_d = nc.dram_tensor(
        "xin", [P, 2 * rows_pad], dt.bfloat16, kind="ExternalInput"
    )
    s_d = nc.dram_tensor("smat", [P, rows_pad], dt.bfloat16, kind="ExternalInput")
    w0_d = nc.dram_tensor("wl0", [P, 4 * P], dt.bfloat16, kind="ExternalInput")
    w1_d = nc.dram_tensor("wl1", [P, 6 * D_OUT], dt.bfloat16, kind="ExternalInput")
    out_d = nc.dram_tensor("out", [n_groups * P, P], dt.float32, kind="ExternalOutput")

    RELU = mybir.ActivationFunctionType.Relu

    with ExitStack() as ctx:
        tc = ctx.enter_context(tile.TileContext(nc))
        wpool = ctx.enter_context(tc.tile_pool(name="w", bufs=1))
        iopool = ctx.enter_context(tc.tile_pool(name="io", bufs=10))
        spool = ctx.enter_context(tc.tile_pool(name="sp", bufs=16))
        mpool = ctx.enter_context(tc.tile_pool(name="mid", bufs=8))
        ypool = ctx.enter_context(tc.tile_pool(name="y", bufs=8))
        fpool = ctx.enter_context(tc.tile_pool(name="fl", bufs=4))
        ps_m = ctx.enter_context(tc.tile_pool(name="psm", bufs=2, space="PSUM"))
        ps_v = ctx.enter_context(tc.tile_pool(name="psv", bufs=1, space="PSUM"))
        ps_l1 = ctx.enter_context(tc.tile_pool(name="psl1", bufs=1, space="PSUM"))
        ps_seg = ctx.enter_context(tc.tile_pool(name="psg", bufs=1, space="PSUM"))

        w0 = wpool.tile([P, 4 * P], dt.bfloat16, tag="w0")
        nc.sync.dma_start(w0[:], w0_d[:, :])
        w1 = wpool.tile([P, 6 * D_OUT], dt.bfloat16, tag="w1")
        nc.sync.dma_start(w1[:], w1_d[:, :])

        seg_ps = None
        for c in range(n_chunks):
            g, cin = divmod(c, chunks_per_group)

            xin = iopool.tile([P, 2 * F], dt.bfloat16, tag="xin")
            nc.sync.dma_start(xin[:], xin_d[:, c * 2 * F : (c + 1) * 2 * F])
            xt = xin[:, 0:F]
            xq = xin[:, F : 2 * F]
            st_t = spool.tile([P, F], dt.bfloat16, tag="st")
            nc.sync.dma_start(st_t[:], s_d[:, c * F : (c + 1) * F])
            st = st_t[:]

            # ---- L0: weights stationary, transposed activations ----
            m0 = ps_m.tile([P, 2 * F], dt.float32, tag="m0")
            v0 = ps_v.tile([P, 2 * F], dt.float32, tag="v0")
            for r in range(2):
                nc.tensor.matmul(
                    m0[:, r * F : (r + 1) * F],
                    lhsT=w0[:, r * P : (r + 1) * P],
                    rhs=xt,
                    start=True,
                    stop=True,
                )
            for r in range(2):
                nc.tensor.matmul(
                    v0[:, r * F : (r + 1) * F],
                    lhsT=w0[:, (2 + r) * P : (3 + r) * P],
                    rhs=xq,
                    start=True,
                    stop=True,
                )

            # ---- gate ----
            m = mpool.tile([P, 2 * F], dt.bfloat16, tag="m")
            msq = mpool.tile([P, 2 * F], dt.bfloat16, tag="msq")
            v = mpool.tile([P, 2 * F], dt.bfloat16, tag="v")
            nc.scalar.activation(m[:], m0[:], RELU)
            nc.scalar.square(msq[:], m[:])
            nc.vector.scalar_tensor_tensor(
                out=v[:],
                in0=m[:],
                scalar=0.0,
                in1=v0[:],
                op0=mybir.AluOpType.is_gt,
                op1=mybir.AluOpType.mult,
            )

            # ---- L1: data stationary -> natural [rows, 64] outputs ----
            l1 = ps_l1.tile([P, F], dt.float32, tag="l1")
            for s in range(4):
                msl = slice(s * P, s * P + D_OUT)
                vsl = slice(s * P + D_OUT, (s + 1) * P)
                for k in range(2):
                    nc.tensor.matmul(
                        l1[:, msl],
                        lhsT=m[:, k * F + s * P : k * F + (s + 1) * P],
                        rhs=w1[:, k * D_OUT : (k + 1) * D_OUT],
                        start=(k == 0),
                        stop=(k == 1),
                    )
                for j, (src2, wofs) in enumerate(
                    [(v, 2), (v, 3), (msq, 4), (msq, 5)]
                ):
                    k = j % 2
                    nc.tensor.matmul(
                        l1[:, vsl],
                        lhsT=src2[:, k * F + s * P : k * F + (s + 1) * P],
                        rhs=w1[:, wofs * D_OUT : (wofs + 1) * D_OUT],
                        start=(j == 0),
                        stop=(j == 3),
                    )

            # ---- epilogue: inv = 1/V1, w = M1*inv, Y = [w | inv] bf16 ----
            l1v = l1[:].rearrange("p (s t) -> p s t", t=P)
            inv = ypool.tile([P, 4 * D_OUT], dt.float32, tag="inv")
            invv = inv[:].rearrange("p (s t) -> p s t", t=D_OUT)
            ynat = ypool.tile([P, F], dt.bfloat16, tag="ynat")
            yv = ynat[:].rearrange("p (s t) -> p s t", t=P)
            nc.vector.reciprocal_approx_fast(
                out=invv[:, :, :], in_=l1v[:, :, D_OUT:P]
            )
            nc.vector.tensor_tensor(
                out=yv[:, :, 0:D_OUT],
                in0=l1v[:, :, 0:D_OUT],
                in1=invv[:, :, :],
                op=mybir.AluOpType.mult,
            )
            nc.vector.tensor_copy(yv[:, :, D_OUT:P], invv[:, :, :])

            # ---- segment reduce: S^T @ Y accumulated over the group ----
            if cin == 0:
                seg_ps = ps_seg.tile([P, P], dt.float32, tag="seg")
            for s in range(4):
                nc.tensor.matmul(
                    seg_ps[:, :],
                    lhsT=st[:, s * P : (s + 1) * P],
                    rhs=ynat[:, s * P : (s + 1) * P],
                    start=(cin == 0 and s == 0),
                    stop=(cin == chunks_per_group - 1 and s == 3),
                )
            if cin == chunks_per_group - 1:
                fl = fpool.tile([P, P], dt.float32, tag="fl")
                nc.vector.tensor_copy(fl[:], seg_ps[:])
                nc.sync.dma_start(out_d[g * P : (g + 1) * P, :], fl[:])

    nc.compile()
    return nc


def _host_prep(X, X_idx, W_mu0, W_lv0, W_mu1, W_lv1):
    """Build per-core input maps + group bases. Returns (in_maps, bases, geom)."""
    X = np.asarray(X, dtype=np.float32)
    idx_all = np.asarray(X_idx).astype(np.int64)
    W_mu0 = np.asarray(W_mu0, dtype=np.float32)
    W_lv0 = np.asarray(W_lv0, dtype=np.float32)
    W_mu1 = np.asarray(W_mu1, dtype=np.float32)
    W_lv1 = np.asarray(W_lv1, dtype=np.float32)

    Wvar0 = np.exp(W_lv0)
    Wvar1 = np.exp(W_lv1)
    A1 = W_mu1 * W_mu1 + Wvar1
    B1 = Wvar1

    w0 = np.concatenate(
        [W_mu0[:, :P], W_mu0[:, P:], Wvar0[:, :P], Wvar0[:, P:]], axis=1
    ).astype(BF16)  # [128, 512]
    w1 = np.concatenate(
        [W_mu1[:P], W_mu1[P:], A1[:P], A1[P:], B1[:P], B1[P:]], axis=1
    ).astype(BF16)  # [128, 384]

    idx_shards = [idx_all[i * SHARD : (i + 1) * SHARD] for i in range(N_CORES)]

    # pick the largest safe group size (subchunks per 128-seg window)
    group_subs = 16
    while group_subs > 1 and not _choose_grouping(idx_shards, group_subs):
        group_subs //= 2
    rows_per_group = group_subs * P
    chunks_per_group = max(1, (group_subs + 3) // 4)
    group_subs = chunks_per_group * 4
    rows_per_group = group_subs * P
    n_groups = (SHARD + rows_per_group - 1) // rows_per_group
    n_chunks = n_groups * chunks_per_group
    rows_pad = n_chunks * F

    in_maps = []
    bases = []
    for i in range(N_CORES):
        xs = X[i * SHARD : (i + 1) * SHARD]  # [62500, 128]
        idx = idx_shards[i]

        xt = np.zeros((P, rows_pad), dtype=BF16)
        xq = np.zeros((P, rows_pad), dtype=BF16)
        xsT = np.ascontiguousarray(xs.T)
        xt[:, :SHARD] = xsT.astype(BF16)
        xq[:, :SHARD] = np.square(xsT).astype(BF16)
        # pad rows: replicate row 0 (keeps V1 > 0; S stays zero there)
        if rows_pad > SHARD:
            xt[:, SHARD:] = xt[:, 0:1]
            xq[:, SHARD:] = xq[:, 0:1]

        # group bases + one-hot S
        gb = np.zeros(n_groups, dtype=np.int64)
        smat = np.zeros((P, rows_pad), dtype=BF16)
        r = np.arange(SHARD)
        grp = r // rows_per_group
        first = np.searchsorted(grp, np.arange(n_groups), side="left")
        for g in range(n_groups):
            if first[g] < SHARD:
                gb[g] = idx[first[g]]
        rel = idx - gb[grp]
        if rel.min() < 0 or rel.max() >= P:
            raise RuntimeError("segment window overflow — grouping invalid")
        # S_host[p, sub*128 + rel] = 1 for row r = sub*128 + p
        sub = r // P
        pp = r % P
        smat[pp, sub * P + rel] = BF16(1.0)

        # interleave per chunk: [xt | xq] so X loads are one DMA
        xin = np.empty((P, 2 * rows_pad), dtype=BF16)
        x3 = xin.reshape(P, n_chunks, 2, F)
        x3[:, :, 0, :] = xt.reshape(P, n_chunks, F)
        x3[:, :, 1, :] = xq.reshape(P, n_chunks, F)

        in_maps.append({"xin": xin, "smat": smat, "wl0": w0, "wl1": w1})
        bases.append(gb)

    geom = dict(
        n_chunks=n_chunks,
        chunks_per_group=chunks_per_group,
        n_groups=n_groups,
    )
    return in_maps, bases, geom


_PROGRAM_CACHE = {}


def kernel(X, X_idx, W_mu0, W_lv0, W_mu1, W_lv1):
    from concourse.bass_utils import run_bass_kernel_spmd

    in_maps, bases, geom = _host_prep(X, X_idx, W_mu0, W_lv0, W_mu1, W_lv1)

    key = tuple(sorted(geom.items()))
    if key not in _PROGRAM_CACHE:
        _PROGRAM_CACHE[key] = _build_program(
            geom["n_chunks"], geom["chunks_per_group"], geom["n_groups"]
        )
    nc = _PROGRAM_CACHE[key]

    res = run_bass_kernel_spmd(nc, in_maps, core_ids=list(range(N_CORES)))
    outs = res.results

    acc = np.zeros((U + P, P), dtype=np.float64)
    for i in range(N_CORES):
        slab = outs[i]["out"].astype(np.float64)  # [n_groups*128, 128]
        gb = bases[i]
        for g in range(geom["n_groups"]):
            acc[gb[g] : gb[g] + P] += slab[g * P : (g + 1) * P]
    acc = acc[:U]

    mean_sum = acc[:, :D_OUT]
    var_inv_sum = acc[:, D_OUT:] + EPS
    emb_var = 1.0 / var_inv_sum
    emb_mean = mean_sum * emb_var
    return (
        emb_mean.astype(np.float32),
        emb_var.astype(np.float32),
    )


# revision 22
# speedup vs baseline: 1.1753x; 1.1753x over previous
"""Trainium2 Bass kernel for DGP-RF embeddings (segment_reduce).

Reference computation (N=500000, D_IN=128, R=256, D_OUT=64, U=10000):
    m0 = X @ Wmu0                      # [N, R]
    v0 = (X*X) @ exp(Wlv0)             # [N, R]
    gate = m0 > 0 ; m = m0*gate ; v = v0*gate
    M1 = m @ Wmu1                      # [N, 64]
    V1 = v @ (Wmu1^2 + exp(Wlv1)) + (m*m) @ exp(Wlv1)
    inv = 1/max(V1, eps)
    var_inv_sum = segment_sum(inv, X_idx, U) + eps
    mean_sum    = segment_sum(M1*inv, X_idx, U)
    emb_var  = 1/var_inv_sum ; emb_mean = mean_sum * emb_var

Strategy (8 cores, data-parallel over rows):
  - Host: shard rows, pre-transpose X to [128, rows] bf16, precompute X^2,
    precompute exp() weight combos, and build the one-hot segment-selector
    matrix S (rows x 128-seg window per group) from the sorted X_idx.
  - Device per 512-row chunk: L0 matmuls (weights stationary, transposed
    activations), ReLU gate (ACT relu / ACT square / DVE copy_predicated),
    L1 matmuls (data stationary -> natural-layout [rows, 64] outputs),
    reciprocal + weighting on DVE, then a segment-reduce matmul with the
    host-built one-hot S as the stationary operand, accumulating 128-segment
    windows in PSUM, flushed per group to DRAM.
  - Host: add per-group 128-seg slabs into full [U, 128] per core, sum
    cores, final divide.
"""

import os
import sys

sys.path.insert(0, "/opt/trn_rl_repo")

import numpy as np
import ml_dtypes

import concourse.bass as bass
import concourse.bacc as bacc
import concourse.mybir as mybir
import concourse.tile as tile
from contextlib import ExitStack

BF16 = ml_dtypes.bfloat16

N, D_IN, R, D_OUT, U = 500000, 128, 256, 64, 10000
EPS = 1e-8
N_CORES = 8
P = 128
F = 512                      # rows per chunk
SHARD = N // N_CORES         # 62500


def _choose_grouping(idx_shards, group_subs):
    """Check that every group of `group_subs` subchunks spans < 128 segments.

    idx_shards: list of per-core int arrays (sorted), real rows only.
    Returns True if grouping is valid.
    """
    rows_per_group = group_subs * P
    for idx in idx_shards:
        n = len(idx)
        for start in range(0, n, rows_per_group):
            seg = idx[start : start + rows_per_group]
            if len(seg) and seg[-1] - seg[0] >= P:
                return False
    return True


def _build_program(n_chunks, chunks_per_group, n_groups):
    dt = mybir.dt
    rows_pad = n_chunks * F
    nc = bacc.Bacc()

    xin_d = nc.dram_tensor(
        "xin", [P, 2 * rows_pad], dt.bfloat16, kind="ExternalInput"
    )
    s_d = nc.dram_tensor("smat", [P, rows_pad], dt.bfloat16, kind="ExternalInput")
    w0_d = nc.dram_tensor("wl0", [P, 4 * P], dt.bfloat16, kind="ExternalInput")
    w1_d = nc.dram_tensor("wl1", [P, 6 * D_OUT], dt.bfloat16, kind="ExternalInput")
    out_d = nc.dram_tensor("out", [n_groups * P, P], dt.float32, kind="ExternalOutput")

    RELU = mybir.ActivationFunctionType.Relu

    with ExitStack() as ctx:
        tc = ctx.enter_context(tile.TileContext(nc))
        wpool = ctx.enter_context(tc.tile_pool(name="w", bufs=1))
        iopool = ctx.enter_context(tc.tile_pool(name="io", bufs=10))
        spool = ctx.enter_context(tc.tile_pool(name="sp", bufs=16))
        mpool = ctx.enter_context(tc.tile_pool(name="mid", bufs=8))
        ypool = ctx.enter_context(tc.tile_pool(name="y", bufs=8))
        fpool = ctx.enter_context(tc.tile_pool(name="fl", bufs=4))
        ps_m = ctx.enter_context(tc.tile_pool(name="psm", bufs=2, space="PSUM"))
        ps_v = ctx.enter_context(tc.tile_pool(name="psv", bufs=1, space="PSUM"))
        ps_l1 = ctx.enter_context(tc.tile_pool(name="psl1", bufs=1, space="PSUM"))
        ps_seg = ctx.enter_context(tc.tile_pool(name="psg", bufs=1, space="PSUM"))

        w0 = wpool.tile([P, 4 * P], dt.bfloat16, tag="w0")
        nc.sync.dma_start(w0[:], w0_d[:, :])
        w1 = wpool.tile([P, 6 * D_OUT], dt.bfloat16, tag="w1")
        nc.sync.dma_start(w1[:], w1_d[:, :])

        seg_ps = None
        for c in range(n_chunks):
            g, cin = divmod(c, chunks_per_group)

            xin = iopool.tile([P, 2 * F], dt.bfloat16, tag="xin")
            nc.sync.dma_start(xin[:], xin_d[:, c * 2 * F : (c + 1) * 2 * F])
            xt = xin[:, 0:F]
            xq = xin[:, F : 2 * F]
            st_t = spool.tile([P, F], dt.bfloat16, tag="st")
            nc.sync.dma_start(st_t[:], s_d[:, c * F : (c + 1) * F])
            st = st_t[:]

            # ---- L0: weights stationary, transposed activations ----
            m0 = ps_m.tile([P, 2 * F], dt.float32, tag="m0")
            v0 = ps_v.tile([P, 2 * F], dt.float32, tag="v0")
            for r in range(2):
                nc.tensor.matmul(
                    m0[:, r * F : (r + 1) * F],
                    lhsT=w0[:, r * P : (r + 1) * P],
                    rhs=xt,
                    start=True,
                    stop=True,
                )
            for r in range(2):
                nc.tensor.matmul(
                    v0[:, r * F : (r + 1) * F],
                    lhsT=w0[:, (2 + r) * P : (3 + r) * P],
                    rhs=xq,
                    start=True,
                    stop=True,
                )

            # ---- gate ----
            m = mpool.tile([P, 2 * F], dt.bfloat16, tag="m")
            msq = mpool.tile([P, 2 * F], dt.bfloat16, tag="msq")
            v = mpool.tile([P, 2 * F], dt.bfloat16, tag="v")
            nc.scalar.activation(m[:], m0[:], RELU)
            nc.scalar.square(msq[:], m[:])
            nc.vector.scalar_tensor_tensor(
                out=v[:],
                in0=m[:],
                scalar=0.0,
                in1=v0[:],
                op0=mybir.AluOpType.is_gt,
                op1=mybir.AluOpType.mult,
            )

            # ---- L1: data stationary -> natural [rows, 64] outputs ----
            l1 = ps_l1.tile([P, F], dt.float32, tag="l1")
            for s in range(4):
                msl = slice(s * P, s * P + D_OUT)
                vsl = slice(s * P + D_OUT, (s + 1) * P)
                for k in range(2):
                    nc.tensor.matmul(
                        l1[:, msl],
                        lhsT=m[:, k * F + s * P : k * F + (s + 1) * P],
                        rhs=w1[:, k * D_OUT : (k + 1) * D_OUT],
                        start=(k == 0),
                        stop=(k == 1),
                    )
                for j, (src2, wofs) in enumerate(
                    [(v, 2), (v, 3), (msq, 4), (msq, 5)]
                ):
                    k = j % 2
                    nc.tensor.matmul(
                        l1[:, vsl],
                        lhsT=src2[:, k * F + s * P : k * F + (s + 1) * P],
                        rhs=w1[:, wofs * D_OUT : (wofs + 1) * D_OUT],
                        start=(j == 0),
                        stop=(j == 3),
                    )

            # ---- epilogue: inv = 1/V1, w = M1*inv, Y = [w | inv] bf16 ----
            l1v = l1[:].rearrange("p (s t) -> p s t", t=P)
            inv = ypool.tile([P, 4 * D_OUT], dt.float32, tag="inv")
            invv = inv[:].rearrange("p (s t) -> p s t", t=D_OUT)
            ynat = ypool.tile([P, F], dt.bfloat16, tag="ynat")
            yv = ynat[:].rearrange("p (s t) -> p s t", t=P)
            nc.vector.reciprocal_approx_fast(
                out=invv[:, :, :], in_=l1v[:, :, D_OUT:P]
            )
            nc.vector.tensor_tensor(
                out=yv[:, :, 0:D_OUT],
                in0=l1v[:, :, 0:D_OUT],
                in1=invv[:, :, :],
                op=mybir.AluOpType.mult,
            )
            nc.vector.tensor_copy(yv[:, :, D_OUT:P], invv[:, :, :])

            # ---- segment reduce: S^T @ Y accumulated over the group ----
            if cin == 0:
                seg_ps = ps_seg.tile([P, P], dt.float32, tag="seg")
            for s in range(4):
                nc.tensor.matmul(
                    seg_ps[:, :],
                    lhsT=st[:, s * P : (s + 1) * P],
                    rhs=ynat[:, s * P : (s + 1) * P],
                    start=(cin == 0 and s == 0),
                    stop=(cin == chunks_per_group - 1 and s == 3),
                )
            if cin == chunks_per_group - 1:
                fl = fpool.tile([P, P], dt.float32, tag="fl")
                nc.scalar.copy(fl[:], seg_ps[:])
                nc.sync.dma_start(out_d[g * P : (g + 1) * P, :], fl[:])

    nc.compile()
    return nc


def _host_prep(X, X_idx, W_mu0, W_lv0, W_mu1, W_lv1):
    """Build per-core input maps + group bases. Returns (in_maps, bases, geom)."""
    X = np.asarray(X, dtype=np.float32)
    idx_all = np.asarray(X_idx).astype(np.int64)
    W_mu0 = np.asarray(W_mu0, dtype=np.float32)
    W_lv0 = np.asarray(W_lv0, dtype=np.float32)
    W_mu1 = np.asarray(W_mu1, dtype=np.float32)
    W_lv1 = np.asarray(W_lv1, dtype=np.float32)

    Wvar0 = np.exp(W_lv0)
    Wvar1 = np.exp(W_lv1)
    A1 = W_mu1 * W_mu1 + Wvar1
    B1 = Wvar1

    w0 = np.concatenate(
        [W_mu0[:, :P], W_mu0[:, P:], Wvar0[:, :P], Wvar0[:, P:]], axis=1
    ).astype(BF16)  # [128, 512]
    w1 = np.concatenate(
        [W_mu1[:P], W_mu1[P:], A1[:P], A1[P:], B1[:P], B1[P:]], axis=1
    ).astype(BF16)  # [128, 384]

    idx_shards = [idx_all[i * SHARD : (i + 1) * SHARD] for i in range(N_CORES)]

    # pick the largest safe group size (subchunks per 128-seg window)
    group_subs = 16
    while group_subs > 1 and not _choose_grouping(idx_shards, group_subs):
        group_subs //= 2
    rows_per_group = group_subs * P
    chunks_per_group = max(1, (group_subs + 3) // 4)
    group_subs = chunks_per_group * 4
    rows_per_group = group_subs * P
    n_groups = (SHARD + rows_per_group - 1) // rows_per_group
    n_chunks = n_groups * chunks_per_group
    rows_pad = n_chunks * F

    in_maps = []
    bases = []
    for i in range(N_CORES):
        xs = X[i * SHARD : (i + 1) * SHARD]  # [62500, 128]
        idx = idx_shards[i]

        xt = np.zeros((P, rows_pad), dtype=BF16)
        xq = np.zeros((P, rows_pad), dtype=BF16)
        xsT = np.ascontiguousarray(xs.T)
        xt[:, :SHARD] = xsT.astype(BF16)
        xq[:, :SHARD] = np.square(xsT).astype(BF16)
        # pad rows: replicate row 0 (keeps V1 > 0; S stays zero there)
        if rows_pad > SHARD:
            xt[:, SHARD:] = xt[:, 0:1]
            xq[:, SHARD:] = xq[:, 0:1]

        # group bases + one-hot S
        gb = np.zeros(n_groups, dtype=np.int64)
        smat = np.zeros((P, rows_pad), dtype=BF16)
        r = np.arange(SHARD)
        grp = r // rows_per_group
        first = np.searchsorted(grp, np.arange(n_groups), side="left")
        for g in range(n_groups):
            if first[g] < SHARD:
                gb[g] = idx[first[g]]
        rel = idx - gb[grp]
        if rel.min() < 0 or rel.max() >= P:
            raise RuntimeError("segment window overflow — grouping invalid")
        # S_host[p, sub*128 + rel] = 1 for row r = sub*128 + p
        sub = r // P
        pp = r % P
        smat[pp, sub * P + rel] = BF16(1.0)

        # interleave per chunk: [xt | xq] so X loads are one DMA
        xin = np.empty((P, 2 * rows_pad), dtype=BF16)
        x3 = xin.reshape(P, n_chunks, 2, F)
        x3[:, :, 0, :] = xt.reshape(P, n_chunks, F)
        x3[:, :, 1, :] = xq.reshape(P, n_chunks, F)

        in_maps.append({"xin": xin, "smat": smat, "wl0": w0, "wl1": w1})
        bases.append(gb)

    geom = dict(
        n_chunks=n_chunks,
        chunks_per_group=chunks_per_group,
        n_groups=n_groups,
    )
    return in_maps, bases, geom


_PROGRAM_CACHE = {}


def kernel(X, X_idx, W_mu0, W_lv0, W_mu1, W_lv1):
    from concourse.bass_utils import run_bass_kernel_spmd

    in_maps, bases, geom = _host_prep(X, X_idx, W_mu0, W_lv0, W_mu1, W_lv1)

    key = tuple(sorted(geom.items()))
    if key not in _PROGRAM_CACHE:
        _PROGRAM_CACHE[key] = _build_program(
            geom["n_chunks"], geom["chunks_per_group"], geom["n_groups"]
        )
    nc = _PROGRAM_CACHE[key]

    res = run_bass_kernel_spmd(nc, in_maps, core_ids=list(range(N_CORES)))
    outs = res.results

    acc = np.zeros((U + P, P), dtype=np.float64)
    for i in range(N_CORES):
        slab = outs[i]["out"].astype(np.float64)  # [n_groups*128, 128]
        gb = bases[i]
        for g in range(geom["n_groups"]):
            acc[gb[g] : gb[g] + P] += slab[g * P : (g + 1) * P]
    acc = acc[:U]

    mean_sum = acc[:, :D_OUT]
    var_inv_sum = acc[:, D_OUT:] + EPS
    emb_var = 1.0 / var_inv_sum
    emb_mean = mean_sum * emb_var
    return (
        emb_mean.astype(np.float32),
        emb_var.astype(np.float32),
    )
